# revision 28
# baseline (speedup 1.0000x reference)
"""Causal multi-head self-attention on 8 Trainium2 cores.

Shapes (hardcoded): x (4, 2048, 1024), H=16 heads, head dim 64.
Sharding: Megatron — core c owns heads 2c, 2c+1 (rows 128c:128c+128 of
Wq/Wk/Wv) for all 4 batches; out-projection row-parallel (Wo columns
128c:128c+128), host sums the 8 partial outputs and adds bo.

Per-core dataflow (per batch b):
  xT[b] (8 d-chunks, 128, 2048) in SBUF
  qT,kT (128=2*64 headdim, 2048) = W-chunk.T @ xT-chunk   (PSUM accum over d)
  V natural (2048, 2*65) with a ones column per head (denominator trick)
  per (head, q-slice of 512): S.T tiles (k128, q512) = kT-slice.T @ qT-slice,
  exp on ACT (scale=1/8 fused), causal via 4 static {0,1} masks on diagonal
  tiles, AV accumulation O.T (65, 512) = V-chunk.T @ P.T — row 64 = softmax
  denominators; normalize via DVE reciprocal + PE broadcast + DVE multiply.
  out partial (s128, e512) = ctxT-slice.T @ WoT-slice.
"""

import os
import sys

import numpy as np

sys.path.insert(0, "/opt/trn_rl_repo")
os.environ.setdefault("MYCRO_LOCAL_CACHE", "1")

B, S, D, H = 4, 2048, 1024, 16
HD = 64          # head dim
NC_ = 8          # cores
P = 128          # partitions
DC = D // P      # 8 d-chunks
SQ = 512         # q column slice
NQ = S // SQ     # 4 q slices
KT = 128         # k tile
NKT = S // KT    # 16 k tiles
NST = S // P     # 16 s tiles

_CACHE = {}


def _build_kernel(pt_dtype_name: str):
    from contextlib import ExitStack

    import concourse.bass as bass  # noqa: F401
    import concourse.mybir as mybir
    import concourse.tile as tile
    from concourse import bacc
    from concourse._compat import with_exitstack

    FP = mybir.dt.float32
    FR = mybir.dt.float32r
    BF = mybir.dt.bfloat16
    PT_DT = {"fr": FR, "bf": BF, "fp": FP}[pt_dtype_name]
    EXPF = mybir.ActivationFunctionType.Exp
    SCALE = 1.0 / np.sqrt(HD)

    @with_exitstack
    def body(ctx: ExitStack, tc, xT, wq, wk, wv, wo, bqv, bkv, bvv, masks,
             identv, onesv, out):
        nc = tc.nc
        singles = ctx.enter_context(tc.tile_pool(name="singles", bufs=1))
        xpool = ctx.enter_context(tc.tile_pool(name="x", bufs=1))
        qkpool = ctx.enter_context(tc.tile_pool(name="qk", bufs=1))
        vpool = ctx.enter_context(tc.tile_pool(name="v", bufs=1))
        ptpool = ctx.enter_context(tc.tile_pool(name="pt", bufs=1))
        ctxpool = ctx.enter_context(tc.tile_pool(name="ctx", bufs=1))
        cpool = ctx.enter_context(tc.tile_pool(name="ctmp", bufs=2))
        rpool = ctx.enter_context(tc.tile_pool(name="recip", bufs=2))
        opool = ctx.enter_context(tc.tile_pool(name="osb", bufs=3))
        psProj = ctx.enter_context(tc.tile_pool(name="psProj", bufs=2, space="PSUM"))
        psST = ctx.enter_context(tc.tile_pool(name="psST", bufs=2, space="PSUM"))
        psAV = ctx.enter_context(tc.tile_pool(name="psAV", bufs=1, space="PSUM"))
        psR = ctx.enter_context(tc.tile_pool(name="psR", bufs=1, space="PSUM"))

        # ---- load constants ----
        wq_sb = singles.tile([P, DC, P], FR)
        wk_sb = singles.tile([P, DC, P], FR)
        wv_sb = singles.tile([P, DC, P], FR)
        for i in range(DC):
            nc.sync.dma_start(wq_sb[:, i, :], wq[i])
            nc.sync.dma_start(wk_sb[:, i, :], wk[i])
            nc.sync.dma_start(wv_sb[:, i, :], wv[i])
        wo_sb = singles.tile([P, D], FR)
        nc.sync.dma_start(wo_sb[:], wo[:])
        masks_sb = singles.tile([P, 4, SQ], FP)
        for j in range(4):
            nc.sync.dma_start(masks_sb[:, j, :], masks[j])
        bq_sb = singles.tile([P, 1], FP)
        bk_sb = singles.tile([P, 1], FP)
        bv_sb = singles.tile([P, 1], FP)
        nc.sync.dma_start(bq_sb[:], bqv[:])
        nc.sync.dma_start(bk_sb[:], bkv[:])
        nc.sync.dma_start(bv_sb[:], bvv[:])

        ones_t = singles.tile([P, HD], FR)
        nc.sync.dma_start(ones_t[:], onesv[:])  # row 64 used as K=1 lhsT
        ident = singles.tile([P, P], FR)
        nc.sync.dma_start(ident[:], identv[:])

        # v natural tile lives across batches; ones columns written once
        v_sb = vpool.tile([P, NST, 130], PT_DT)
        nc.sync.dma_start(v_sb[:, :, 64:65], onesv[:, 0:NST])
        nc.sync.dma_start(v_sb[:, :, 129:130], onesv[:, 0:NST])

        for b in range(B):
            x_sb = xpool.tile([P, DC, S], FR)
            for i in range(DC):
                nc.sync.dma_start(x_sb[:, i, :], xT[b, i])

            # ---- q/k/v projections: (128, S) T-layout ----
            qT = qkpool.tile([P, S], FR)
            kT = qkpool.tile([P, S], FR)
            vT = qkpool.tile([P, S], FR)
            for w_sb, bias, dst in ((wq_sb, bq_sb, qT), (wk_sb, bk_sb, kT),
                                    (wv_sb, bv_sb, vT)):
                for ss in range(NQ):
                    ps = psProj.tile([P, SQ], FP)
                    for i in range(DC):
                        nc.tensor.matmul(
                            ps[:],
                            w_sb[:, i, :],
                            x_sb[:, i, ss * SQ:(ss + 1) * SQ],
                            start=i == 0,
                            stop=i == DC - 1,
                        )
                    nc.vector.tensor_scalar_add(
                        dst[:, ss * SQ:(ss + 1) * SQ], ps[:], bias[:]
                    )

            # ---- v natural (s, 2*(64+1)) via PE transpose ----
            for st in range(NST):
                tps = psProj.tile([P, P], FR, tag="ps")
                nc.tensor.transpose(
                    tps[:], vT[:, st * P:(st + 1) * P], ident[:]
                )
                for h in range(2):
                    nc.vector.tensor_copy(
                        v_sb[:, st, 65 * h:65 * h + 64],
                        tps[:, 64 * h:64 * h + 64],
                    )

            # ---- attention per head ----
            ctxT = ctxpool.tile([P, S], FR)
            for h in range(2):
                bp = 64 * h
                for qt in range(NQ):
                    nkt = 4 * qt + 4
                    pt = ptpool.tile([P, nkt, SQ], PT_DT)
                    for kp in range(nkt // 2):
                        st2 = psST.tile([P, 2, SQ], FP)
                        for jj in range(2):
                            kt = 2 * kp + jj
                            nc.tensor.matmul(
                                st2[:, jj, :],
                                kT[bp:bp + 64, kt * KT:(kt + 1) * KT],
                                qT[bp:bp + 64, qt * SQ:(qt + 1) * SQ],
                                start=True,
                                stop=True,
                            )
                        nc.scalar.activation(
                            pt[:, 2 * kp:2 * kp + 2, :], st2[:], EXPF, scale=SCALE
                        )
                        for jj in range(2):
                            kt = 2 * kp + jj
                            if kt >= 4 * qt:
                                nc.vector.tensor_mul(
                                    pt[:, kt, :],
                                    pt[:, kt, :],
                                    masks_sb[:, kt - 4 * qt, :],
                                )
                    av = psAV.tile([65, SQ], FP)
                    for kt in range(nkt):
                        nc.tensor.matmul(
                            av[:],
                            v_sb[:, kt, 65 * h:65 * h + 65],
                            pt[:, kt, :],
                            start=kt == 0,
                            stop=kt == nkt - 1,
                        )
                    # normalize: rows 0:64 /= row 64
                    recip = rpool.tile([65, SQ], FR)
                    with nc.allow_low_precision(reason="fp32r is fp32-width"):
                        nc.vector.reciprocal(recip[64:65, :], av[64:65, :])
                    rb = psR.tile([HD, SQ], FP)
                    nc.tensor.matmul(
                        rb[:], ones_t[64:65, :], recip[64:65, :],
                        start=True, stop=True,
                    )
                    rb_sb = rpool.tile([HD, SQ], FR)
                    nc.vector.tensor_copy(rb_sb[:], rb[:])
                    if h == 0:
                        nc.vector.tensor_mul(
                            ctxT[0:64, qt * SQ:(qt + 1) * SQ],
                            av[0:64, :], rb_sb[:],
                        )
                    else:
                        ctmp = cpool.tile([64, SQ], FR)
                        nc.vector.tensor_mul(ctmp[:], av[0:64, :], rb_sb[:])
                        nc.gpsimd.dma_start(
                            ctxT[64:128, qt * SQ:(qt + 1) * SQ], ctmp[:]
                        )

            # ---- out projection: partial (s128, e) tiles ----
            for st in range(NST):
                osb = opool.tile([P, D], FP)
                for es in range(2):
                    ps = psProj.tile([P, SQ], FP)
                    nc.tensor.matmul(
                        ps[:],
                        ctxT[:, st * P:(st + 1) * P],
                        wo_sb[:, es * SQ:(es + 1) * SQ],
                        start=True,
                        stop=True,
                    )
                    nc.vector.tensor_copy(osb[:, es * SQ:(es + 1) * SQ], ps[:])
                nc.gpsimd.dma_start(out[b, st], osb[:])

    import concourse.mybir as mybir
    import concourse.tile as tile
    from concourse import bacc

    FP = mybir.dt.float32
    FR = mybir.dt.float32r
    BF = mybir.dt.bfloat16

    nc = bacc.Bacc(None, target_bir_lowering=False, debug=False)
    with tile.TileContext(nc) as tc:
        with tc.tile_pool(name="dram", bufs=1, space="DRAM") as dram:
            xT = dram.tile((B, DC, P, S), FR, kind="ExternalInput")
            wq = dram.tile((DC, P, P), FR, kind="ExternalInput")
            wk = dram.tile((DC, P, P), FR, kind="ExternalInput")
            wv = dram.tile((DC, P, P), FR, kind="ExternalInput")
            wo = dram.tile((P, D), FR, kind="ExternalInput")
            bqv = dram.tile((P, 1), FP, kind="ExternalInput")
            bkv = dram.tile((P, 1), FP, kind="ExternalInput")
            bvv = dram.tile((P, 1), FP, kind="ExternalInput")
            masks = dram.tile((4, P, SQ), FP, kind="ExternalInput")
            identv = dram.tile((P, P), FR, kind="ExternalInput")
            onesv = dram.tile((P, HD), FR, kind="ExternalInput")
            out = dram.tile((B, NST, P, D), FP, kind="ExternalOutput")
            body(tc, xT[:], wq[:], wk[:], wv[:], wo[:], bqv[:], bkv[:],
                 bvv[:], masks[:], identv[:], onesv[:], out[:])
    nc.compile()
    names = dict(xT=xT.name, wq=wq.name, wk=wk.name, wv=wv.name, wo=wo.name,
                 bq=bqv.name, bk=bkv.name, bv=bvv.name, masks=masks.name,
                 ident=identv.name, ones=onesv.name, out=out.name)
    return nc, names


def get_compiled(pt_dtype_name="fr"):
    key = pt_dtype_name
    if key not in _CACHE:
        _CACHE[key] = _build_kernel(pt_dtype_name)
    return _CACHE[key]


def make_in_maps(x, Wq, bq, Wk, bk, Wv, bv, Wo, bo, names):
    f32 = np.float32
    xT = np.ascontiguousarray(x.transpose(0, 2, 1)).reshape(B, DC, P, S)
    xT = xT.astype(f32, copy=False)
    ksl = np.arange(P)[None, :, None]
    qsl = np.arange(SQ)[None, None, :]
    j = np.arange(4)[:, None, None]
    masks = (KT * j + ksl <= qsl).astype(f32)
    ident = np.eye(P, dtype=f32)
    ones = np.ones((P, HD), dtype=f32)
    in_maps = []
    for c in range(NC_):
        r = slice(P * c, P * (c + 1))
        wq_c = np.ascontiguousarray(Wq[r, :].T).reshape(DC, P, P).astype(f32)
        wk_c = np.ascontiguousarray(Wk[r, :].T).reshape(DC, P, P).astype(f32)
        wv_c = np.ascontiguousarray(Wv[r, :].T).reshape(DC, P, P).astype(f32)
        wo_c = np.ascontiguousarray(Wo[:, r].T).astype(f32)
        in_maps.append({
            names["xT"]: xT,
            names["wq"]: wq_c,
            names["wk"]: wk_c,
            names["wv"]: wv_c,
            names["wo"]: wo_c,
            names["bq"]: bq[r].reshape(P, 1).astype(f32),
            names["bk"]: bk[r].reshape(P, 1).astype(f32),
            names["bv"]: bv[r].reshape(P, 1).astype(f32),
            names["masks"]: masks,
            names["ident"]: ident,
            names["ones"]: ones,
        })
    return in_maps


def kernel(x, Wq, bq, Wk, bk, Wv, bv, Wo, bo):
    from concourse.bass_utils import run_bass_kernel_spmd

    nc, names = get_compiled()
    in_maps = make_in_maps(x, Wq, bq, Wk, bk, Wv, bv, Wo, bo, names)
    res = run_bass_kernel_spmd(nc, in_maps, core_ids=list(range(NC_)))
    total = np.zeros((B, S, D), np.float32)
    for c in range(NC_):
        total += res.results[c][names["out"]].reshape(B, S, D)
    total += np.asarray(bo, np.float32)[None, None, :]
    return total
